# revision 26
# baseline (speedup 1.0000x reference)
"""Trainium2 Bass kernel for the bidirectional Mamba MixerModel problem.

fp16 on everything matmuls touch (weights + activations + the out-projection
AllReduce); fp32 kept on the precision-critical paths (softplus/dt chain, LN
stats, PSUM accumulators, x_dbl AllReduce).  The selective scan runs as
merged 1024-token tensor_tensor_scan ops per state with tiny fp32 carries;
the per-state C-multiply runs on the otherwise-idle GpSimd engine (PSUM
evacuated to SBUF by the Scalar engine first, since GpSimd has no PSUM
port).  Emission is software-pipelined: the next block/batch's LayerNorm +
in-projection + conv chunks are interleaved into the scan loop at fixed
(half, state) slots so the in-order engine queues overlap them with the
scan, with dedicated PSUM pools per phase so bank reuse can't serialize.
The inter-block sequence flip is folded into reversed write APs of the
out-projection evacuation.

Sharding: tensor-parallel over d_inner (128 channels per core, 8 cores),
AllReduce for x_dbl (fp32) and the out-projection partials (fp16) per
block/batch.
"""
import os
import sys
import numpy as np

sys.path.insert(0, "/opt/trn_rl_repo")

import ml_dtypes  # noqa: E402

import concourse.bass as bass  # noqa: E402,F401
import concourse.bacc as bacc  # noqa: E402
import concourse.tile as tile  # noqa: E402
from concourse import mybir  # noqa: E402
from concourse import bass_utils  # noqa: E402

F32 = mybir.dt.float32
F16 = mybir.dt.float16
F32R = mybir.dt.float32r
Alu = mybir.AluOpType
Act = mybir.ActivationFunctionType

B, L, D, DI = 2, 2048, 512, 1024
NST, KCONV, RDT, NB = 16, 4, 32, 4
NCORES = 8
DS = DI // NCORES          # 128 channels per core
T = B * L                  # 4096 tokens
CH = 512                   # token chunk for LN/in-proj (1 PSUM bank fp32)
NCH = L // CH              # 4 chunks per batch
HW = 1024                  # token half for the scan phase
NHW = L // HW              # 2 halves per batch
NG = D // 128              # 4 partition groups of the model dim
EPS = 1e-5
POOL_SCAN = False          # Pool-engine scan fails the TRN2 ISA opcode check

_PROGRAM_CACHE = {}


def _build_program(has_lnb: bool, has_nfb: bool):
    nc = bacc.Bacc("TRN2", target_bir_lowering=False, debug=False,
                   enable_asserts=False, num_devices=NCORES)

    tensors = {}
    tensors["xT"] = nc.dram_tensor("xT", [D, T], F16, kind="ExternalInput")
    tensors["wi"] = nc.dram_tensor("wi", [NB, 128, 1024], F16,
                                   kind="ExternalInput")
    tensors["negrs"] = nc.dram_tensor("negrs", [NB, 1, 256], F16,
                                      kind="ExternalInput")
    tensors["biasin"] = nc.dram_tensor("biasin", [NB, 128, 2], F32,
                                       kind="ExternalInput")
    tensors["convd"] = nc.dram_tensor("convd", [NB, 128, KCONV * 128], F16,
                                      kind="ExternalInput")
    tensors["convb"] = nc.dram_tensor("convb", [NB, 128, 1], F32,
                                      kind="ExternalInput")
    tensors["wxT"] = nc.dram_tensor("wxT", [NB, 128, 64], F16,
                                    kind="ExternalInput")
    tensors["wdtT"] = nc.dram_tensor("wdtT", [NB, 32, 128], F16,
                                     kind="ExternalInput")
    tensors["bdt"] = nc.dram_tensor("bdt", [NB, 128, 1], F32,
                                    kind="ExternalInput")
    tensors["acols"] = nc.dram_tensor("acols", [NB, 128, NST], F32,
                                      kind="ExternalInput")
    tensors["dpd"] = nc.dram_tensor("dpd", [NB, 128, 1], F32,
                                    kind="ExternalInput")
    tensors["woT"] = nc.dram_tensor("woT", [NB, 128, 512], F16,
                                    kind="ExternalInput")
    tensors["nfw"] = nc.dram_tensor("nfw", [128, NG], F32,
                                    kind="ExternalInput")
    tensors["nfb"] = nc.dram_tensor("nfb", [128, NG], F32,
                                    kind="ExternalInput")
    tensors["identin"] = nc.dram_tensor("identin", [128, 128], F16,
                                        kind="ExternalInput")
    tensors["selbc"] = nc.dram_tensor("selbc", [64, 32 * 128], F16,
                                      kind="ExternalInput")
    tensors["onesin"] = nc.dram_tensor("onesin", [128, 128], F16,
                                       kind="ExternalInput")
    tensors["outT"] = nc.dram_tensor("outT", [D, T], F32,
                                     kind="ExternalOutput")

    xdbl_in, xdbl_out, op_in, op_out = [], [], [], []
    for i in range(NB):
        xi_b, xo_b, oi_b, oo_b = [], [], [], []
        for b in range(B):
            xi_b.append(nc.dram_tensor(f"xdbl_in_{i}_{b}", [64, L], F32,
                                       kind="Internal"))
            xo_b.append(nc.dram_tensor(f"xdbl_out_{i}_{b}", [64, L], F32,
                                       kind="Internal", addr_space="Shared"))
            oi_b.append(nc.dram_tensor(f"op_in_{i}_{b}", [D, L], F16,
                                       kind="Internal"))
            oo_b.append(nc.dram_tensor(f"op_out_{i}_{b}", [D, L], F16,
                                       kind="Internal", addr_space="Shared"))
        xdbl_in.append(xi_b); xdbl_out.append(xo_b)
        op_in.append(oi_b); op_out.append(oo_b)
    tensors["xdbl_in"], tensors["xdbl_out"] = xdbl_in, xdbl_out
    tensors["op_in"], tensors["op_out"] = op_in, op_out

    with tile.TileContext(nc) as tc:
        _emit(nc, tc, tensors, has_lnb, has_nfb)

    nc.compile()
    return nc


def _emit(nc, tc, Tn, has_lnb, has_nfb):
    import contextlib
    RG = [list(range(NCORES))]
    xdbl_in, xdbl_out = Tn["xdbl_in"], Tn["xdbl_out"]
    op_in, op_out = Tn["op_in"], Tn["op_out"]

    ctx = contextlib.ExitStack()
    with ctx:
        consts = ctx.enter_context(tc.tile_pool(name="consts", bufs=1))
        wpool = ctx.enter_context(tc.tile_pool(name="wpool", bufs=2))
        xin = ctx.enter_context(tc.tile_pool(name="xin", bufs=5))
        small = ctx.enter_context(tc.tile_pool(name="small", bufs=2))
        stats = ctx.enter_context(tc.tile_pool(name="stats", bufs=5))
        bigs = ctx.enter_context(tc.tile_pool(name="bigs", bufs=1))
        spool = ctx.enter_context(tc.tile_pool(name="spool", bufs=3))
        evac = ctx.enter_context(tc.tile_pool(name="evac", bufs=2))
        # PSUM 8 banks: ab(2) for phases A/B/wx/final-LN, mm(2) for dt/op,
        # bc(2) for B/C broadcasts, y(2) for the scan accumulator.  Separate
        # pools keep next-block stats from serializing behind the scan phase.
        ps_ab = ctx.enter_context(tc.tile_pool(name="ps_ab", bufs=2,
                                               space="PSUM"))
        ps_mm = ctx.enter_context(tc.tile_pool(name="ps_mm", bufs=2,
                                               space="PSUM"))
        ps_bc = ctx.enter_context(tc.tile_pool(name="ps_bc", bufs=4,
                                               space="PSUM"))

        ident = consts.tile([128, 128], F16, tag="ident")
        nc.sync.dma_start(out=ident[:], in_=Tn["identin"].ap())
        onesbf = consts.tile([128, 128], F16, tag="onesbf")
        nc.sync.dma_start(out=onesbf[:], in_=Tn["onesin"].ap())
        ones1r = consts.tile([1, 128], F32R, tag="ones1r")
        nc.vector.memset(ones1r[:].bitcast(F32), 1.0)
        nfw_sb = consts.tile([128, NG], F32, tag="nfw")
        nc.sync.dma_start(out=nfw_sb[:], in_=Tn["nfw"].ap())
        nfb_sb = consts.tile([128, NG], F32, tag="nfb")
        nc.sync.dma_start(out=nfb_sb[:], in_=Tn["nfb"].ap())
        eps_sb = consts.tile([128, 1], F32, tag="eps")
        nc.vector.memset(eps_sb[:], EPS)
        selbc_sb = consts.tile([64, 32 * 128], F16, tag="selbc")
        nc.sync.dma_start(out=selbc_sb[:], in_=Tn["selbc"].ap())

        onescol = onesbf[:, 0:1]     # [128,1] bf16 lhsT for stats
        ones1 = onesbf[0:1, :]       # [1,128] bf16 lhsT for broadcasts

        def mm(out, lhsT, rhs, **kw):
            nc.tensor.matmul(out, lhsT=lhsT, rhs=rhs, **kw)

        def src_ap(i, b, g, t0, t1):
            """Block-i input (already flipped), batch b, feature group g."""
            if i == 0:
                return Tn["xT"].ap()[128 * g:128 * (g + 1),
                                     b * L + t0: b * L + t1]
            return op_out[i - 1][b].ap()[128 * g:128 * (g + 1), t0:t1]

        def ln_stats(st_ps, xg_tiles):
            # st_ps: [128, CH] psum tile; partition 0 = sum x, 32 = sum x^2
            for g in range(NG):
                xsq = small.tile([128, CH], F16, tag="xsq")
                nc.scalar.square(out=xsq[:], in_=xg_tiles[g][:])
                mm(st_ps[0:1, :], lhsT=onescol, rhs=xg_tiles[g][:],
                   start=(g == 0), stop=(g == NG - 1), skip_group_check=True)
                mm(st_ps[32:33, :], lhsT=onescol, rhs=xsq[:],
                   start=(g == 0), stop=(g == NG - 1), skip_group_check=True)

        def load_weights(i):
            W = {}
            W["wi"] = wpool.tile([128, 1024], F16, tag="wi", name="wi")
            nc.sync.dma_start(out=W["wi"][:], in_=Tn["wi"].ap()[i])
            W["negrs"] = wpool.tile([1, 256], F16, tag="negrs", name="negrs")
            nc.sync.dma_start(out=W["negrs"][:], in_=Tn["negrs"].ap()[i])
            W["convd"] = wpool.tile([128, KCONV * 128], F16, tag="convd", name="convd")
            nc.sync.dma_start(out=W["convd"][:], in_=Tn["convd"].ap()[i])
            W["convb"] = wpool.tile([128, 1], F32, tag="convb", name="convb")
            nc.sync.dma_start(out=W["convb"][:], in_=Tn["convb"].ap()[i])
            W["wx"] = wpool.tile([128, 64], F16, tag="wx", name="wx")
            nc.sync.dma_start(out=W["wx"][:], in_=Tn["wxT"].ap()[i])
            W["wdt"] = wpool.tile([32, 128], F16, tag="wdt", name="wdt")
            nc.sync.dma_start(out=W["wdt"][:], in_=Tn["wdtT"].ap()[i])
            W["bdt"] = wpool.tile([128, 1], F32, tag="bdt", name="bdt")
            nc.sync.dma_start(out=W["bdt"][:], in_=Tn["bdt"].ap()[i])
            W["acols"] = wpool.tile([128, NST], F32, tag="acols", name="acols")
            nc.sync.dma_start(out=W["acols"][:], in_=Tn["acols"].ap()[i])
            W["dpd"] = wpool.tile([128, 1], F32, tag="dpd", name="dpd")
            nc.sync.dma_start(out=W["dpd"][:], in_=Tn["dpd"].ap()[i])
            W["wo"] = wpool.tile([128, 512], F16, tag="wo", name="wo")
            nc.sync.dma_start(out=W["wo"][:], in_=Tn["woT"].ap()[i])
            if has_lnb:
                W["biasin"] = wpool.tile([128, 2], F32, tag="biasin", name="biasin")
                nc.sync.dma_start(out=W["biasin"][:],
                                  in_=Tn["biasin"].ap()[i])
            return W

        def ab_start(b):
            st = {}
            st["xipad"] = bigs.tile([128, L + 3], F16, tag=f"xipad{b}",
                                    name=f"xipad{b}")
            st["xi"] = bigs.tile([128, L], F16, tag=f"xibf{b}",
                                 name=f"xibf{b}")
            st["sz"] = bigs.tile([128, L], F16, tag=f"sz{b}", name=f"sz{b}")
            st["varall"] = stats.tile([128, CH], F32, tag="varall",
                                      name="varall")
            st["rstd"] = stats.tile([128, CH], F16, tag="rstdall",
                                    name="rstdall")
            st["s1"] = {}
            nc.vector.memset(st["varall"][:], 1.0)
            nc.vector.memset(st["xipad"][:, 0:3], 0.0)
            return st

        def a_chunk(i, b, W, st, c):
            t0 = c * CH
            xg_tiles = []
            for g in range(NG):
                xg = xin.tile([128, CH], F16, tag="xg")
                nc.sync.dma_start(out=xg[:],
                                  in_=src_ap(i, b, g, t0, t0 + CH))
                xg_tiles.append(xg)
            st_ps = ps_ab.tile([128, CH], F32, tag="ab")
            ln_stats(st_ps, xg_tiles)
            s1_row = stats.tile([1, CH], F16, tag="s1", name="s1_row")
            nc.scalar.copy(out=s1_row[:], in_=st_ps[0:1, :])
            st["s1"][c] = s1_row
            mu2 = small.tile([1, CH], F32, tag="mu2")
            nc.scalar.activation(out=mu2[:], in_=st_ps[0:1, :],
                                 func=Act.Square, scale=1.0 / D)
            nc.vector.scalar_tensor_tensor(
                out=st["varall"][32 * c:32 * c + 1, :], in0=st_ps[32:33, :],
                scalar=1.0 / D, in1=mu2[:], op0=Alu.mult, op1=Alu.subtract)

        def rsqrt_batch(st):
            # one Ln-table round-trip for all four chunks' rstd rows
            nc.scalar.activation(out=st["varall"][:], in_=st["varall"][:],
                                 func=Act.Ln, bias=eps_sb[:])
            nc.scalar.activation(out=st["rstd"][:], in_=st["varall"][:],
                                 func=Act.Exp, scale=-0.5)

        def b_chunk(i, b, W, st, c):
            t0 = c * CH
            xg_tiles = []
            for g in range(NG):
                xg = xin.tile([128, CH], F16, tag="xg")
                nc.sync.dma_start(out=xg[:],
                                  in_=src_ap(i, b, g, t0, t0 + CH))
                xg_tiles.append(xg)
            rstd_row = small.tile([1, CH], F16, tag="rstds",
                                  name="rstd_row")
            nc.scalar.copy(out=rstd_row[:],
                           in_=st["rstd"][32 * c:32 * c + 1, :])
            rbc_ps = ps_ab.tile([128, CH], F32, tag="ab")
            mm(rbc_ps[:], lhsT=ones1, rhs=rstd_row[:], start=True, stop=True)
            rbc = small.tile([128, CH], F16, tag="rbc")
            nc.scalar.copy(out=rbc[:], in_=rbc_ps[:])
            for grp in range(2):  # 0 = xi, 1 = z
                xz_ps = ps_ab.tile([128, CH], F32, tag="ab")
                for k in range(4):
                    lh = W["wi"][:, (grp * 4 + k) * 128:
                                 (grp * 4 + k + 1) * 128]
                    mm(xz_ps[:], lhsT=lh, rhs=xg_tiles[k][:],
                       start=(k == 0), stop=False)
                mm(xz_ps[:], lhsT=W["negrs"][:, grp * 128:(grp + 1) * 128],
                   rhs=st["s1"][c][:], start=False, stop=True)
                if grp == 0:
                    dest = st["xipad"][:, 3 + t0: 3 + t0 + CH]
                else:
                    dest = st["sz"][:, t0: t0 + CH]
                nc.vector.tensor_mul(out=dest, in0=xz_ps[:], in1=rbc[:])
                if has_lnb:
                    nc.vector.tensor_scalar_add(
                        out=dest, in0=dest,
                        scalar1=W["biasin"][:, grp:grp + 1])
            cv_ps = ps_ab.tile([128, CH], F32, tag="ab")
            for kk in range(KCONV):
                mm(cv_ps[:], lhsT=W["convd"][:, kk * 128:(kk + 1) * 128],
                   rhs=st["xipad"][:, t0 + kk: t0 + kk + CH],
                   start=(kk == 0), stop=(kk == KCONV - 1))
            nc.scalar.activation(out=st["xi"][:, t0:t0 + CH], in_=cv_ps[:],
                                 func=Act.Identity, bias=W["convb"][:])

        def cwx(i, b, W, st):
            """silu + Wx projection + x_dbl AllReduce for one batch."""
            nc.scalar.activation(out=st["xi"][:], in_=st["xi"][:],
                                 func=Act.Silu)
            nc.scalar.activation(out=st["sz"][:], in_=st["sz"][:],
                                 func=Act.Silu)
            for c in range(NCH):
                t0 = c * CH
                wx_ps = ps_ab.tile([128, CH], F32, tag="ab")
                mm(wx_ps[0:64, :], lhsT=W["wx"][:],
                   rhs=st["xi"][:, t0:t0 + CH], start=True, stop=True,
                   skip_group_check=True)
                wxe = small.tile([64, CH], F32, tag="wxe", name="wxe")
                nc.scalar.copy(out=wxe[:], in_=wx_ps[0:64, :])
                nc.sync.dma_start(out=xdbl_in[i][b].ap()[:, t0:t0 + CH],
                                  in_=wxe[:])
            nc.gpsimd.collective_compute(
                "AllReduce", Alu.add, replica_groups=RG,
                ins=[xdbl_in[i][b].ap()], outs=[xdbl_out[i][b].ap()])

        def fln_chunk(b, c):
            """Final layernorm for one 512-token chunk."""
            t0 = c * CH
            xg_tiles = []
            for g in range(NG):
                xg = xin.tile([128, CH], F16, tag="xg")
                nc.sync.dma_start(out=xg[:],
                                  in_=src_ap(NB, b, g, t0, t0 + CH))
                xg_tiles.append(xg)
            st_ps = ps_ab.tile([128, CH], F32, tag="ab")
            ln_stats(st_ps, xg_tiles)
            m_row = small.tile([1, CH], F32R, tag="m_row")
            nc.vector.tensor_scalar_mul(out=m_row[:], in0=st_ps[0:1, :],
                                        scalar1=1.0 / D)
            mu2 = small.tile([1, CH], F32, tag="mu2")
            nc.vector.tensor_mul(out=mu2[:], in0=m_row[:].bitcast(F32),
                                 in1=m_row[:].bitcast(F32))
            var_row = small.tile([1, CH], F32, tag="var")
            nc.vector.scalar_tensor_tensor(
                out=var_row[:], in0=st_ps[32:33, :], scalar=1.0 / D,
                in1=mu2[:], op0=Alu.mult, op1=Alu.subtract)
            rstd_row = small.tile([1, CH], F32R, tag="rstdf",
                                  name="rstd_row")
            nc.scalar.activation(out=var_row[:], in_=var_row[:],
                                 func=Act.Ln, bias=eps_sb[:1, :])
            nc.scalar.activation(out=rstd_row[:], in_=var_row[:],
                                 func=Act.Exp, scale=-0.5)
            mbc_ps = ps_ab.tile([128, CH], F32, tag="ab")
            mm(mbc_ps[:], lhsT=ones1r[:], rhs=m_row[:], start=True, stop=True)
            rbc_ps = ps_ab.tile([128, CH], F32, tag="ab")
            mm(rbc_ps[:], lhsT=ones1r[:], rhs=rstd_row[:],
               start=True, stop=True)
            rbc = small.tile([128, CH], F32, tag="rbcf")
            nc.scalar.copy(out=rbc[:], in_=rbc_ps[:])
            for g in range(NG):
                t1_sb = small.tile([128, CH], F32, tag="xsqf", name="t1_sb")
                nc.vector.tensor_sub(out=t1_sb[:], in0=xg_tiles[g][:],
                                     in1=mbc_ps[:])
                o_sb = evac.tile([128, CH], F32, tag="ogf", name="o_sb")
                nc.vector.scalar_tensor_tensor(
                    out=o_sb[:], in0=t1_sb[:], scalar=nfw_sb[:, g:g + 1],
                    in1=rbc[:], op0=Alu.mult, op1=Alu.mult)
                if has_nfb:
                    nc.vector.tensor_scalar_add(
                        out=o_sb[:], in0=o_sb[:], scalar1=nfb_sb[:, g:g + 1])
                nc.sync.dma_start(
                    out=Tn["outT"].ap()[g * 128:(g + 1) * 128,
                                        b * L + t0: b * L + t0 + CH],
                    in_=o_sb[:])

        Wq = {}
        stq = {}

        def phase_d(i, b, sched):
            """Softplus dt + merged scans + out-proj for one batch, with
            pending next-phase work interleaved at fixed (h, n) slots."""
            W, st = Wq[i], stq[(i, b)]
            xdbl_sb = bigs.tile([64, L], F32, tag=f"xdbl{b}",
                                name=f"xdbl{b}")
            nc.sync.dma_start(out=xdbl_sb[:], in_=xdbl_out[i][b].ap())
            xdbl_bf = bigs.tile([64, L], F16, tag=f"xdblbf{b}",
                                name=f"xdblbf{b}")
            nc.scalar.copy(out=xdbl_bf[:], in_=xdbl_sb[:])

            dt = bigs.tile([128, L], F32, tag=f"dt{b}", name=f"dt{b}")
            for c in range(NCH):
                t0 = c * CH
                dt_ps = ps_mm.tile([128, CH], F32, tag="mm", name="dt_ps")
                mm(dt_ps[:], lhsT=W["wdt"][:],
                   rhs=xdbl_bf[0:32, t0:t0 + CH], start=True, stop=True)
                nc.scalar.activation(out=dt[:, t0:t0 + CH], in_=dt_ps[:],
                                     func=Act.Identity, bias=W["bdt"][:])
            esb = bigs.tile([128, L], F16, tag=f"esb{b}", name=f"esb{b}")
            nc.scalar.activation(out=esb[:], in_=dt[:], func=Act.Exp,
                                 scale=-1.0)
            nc.scalar.activation(out=esb[:], in_=esb[:], func=Act.Ln,
                                 bias=1.0)
            nc.vector.tensor_add(out=dt[:], in0=dt[:], in1=esb[:])
            dtx = bigs.tile([128, L], F16, tag=f"dtx{b}", name=f"dtx{b}")
            nc.vector.tensor_mul(out=dtx[:], in0=dt[:], in1=st["xi"][:])

            carry = small.tile([128, NST], F32, tag="carry")
            for h in range(NHW):
                t0 = h * HW
                y_sb = evac.tile([128, HW], F32, tag="ysb", name="y_sb")
                for n in range(NST):
                    if (h, n) in sched:
                        sched[(h, n)]()
                    dA = spool.tile([128, HW], F16, tag="dA")
                    nc.scalar.activation(out=dA[:], in_=dt[:, t0:t0 + HW],
                                         func=Act.Exp,
                                         scale=W["acols"][:, n:n + 1])
                    dBu = spool.tile([128, HW], F16, tag="dBu")
                    for q in range(2):
                        bbc_ps = ps_bc.tile([128, CH], F32, tag="bc")
                        mm(bbc_ps[:],
                           lhsT=selbc_sb[:, n * 128:(n + 1) * 128],
                           rhs=xdbl_bf[:, t0 + q * CH: t0 + (q + 1) * CH],
                           start=True, stop=True)
                        nc.vector.tensor_mul(
                            out=dBu[:, q * CH:(q + 1) * CH],
                            in0=dtx[:, t0 + q * CH: t0 + (q + 1) * CH],
                            in1=bbc_ps[:])
                    hsc = spool.tile([128, HW], F16, tag="h")
                    init = 0.0 if h == 0 else carry[:, n:n + 1]
                    nc.vector.tensor_tensor_scan(hsc[:], dA[:], dBu[:],
                                                 init, op0=Alu.mult,
                                                 op1=Alu.add)
                    if h < NHW - 1:
                        nc.scalar.copy(out=carry[:, n:n + 1],
                                       in_=hsc[:, HW - 1:HW])
                    for q in range(2):
                        cbc_ps = ps_bc.tile([128, CH], F32, tag="bc")
                        mm(cbc_ps[:],
                           lhsT=selbc_sb[:, (16 + n) * 128:(17 + n) * 128],
                           rhs=xdbl_bf[:, t0 + q * CH: t0 + (q + 1) * CH],
                           start=True, stop=True)
                        cbc_sb = spool.tile([128, CH], F16, tag="cbcsb")
                        nc.scalar.copy(out=cbc_sb[:], in_=cbc_ps[:])
                        yterm = spool.tile([128, CH], F16, tag="yterm")
                        nc.gpsimd.tensor_mul(
                            out=yterm[:], in0=hsc[:, q * CH:(q + 1) * CH],
                            in1=cbc_sb[:])
                        ysl = y_sb[:, q * CH:(q + 1) * CH]
                        if n == 0:
                            nc.vector.tensor_copy(out=ysl, in_=yterm[:])
                        elif n % 2 == 0:
                            nc.vector.tensor_add(out=ysl, in0=ysl,
                                                 in1=yterm[:])
                        else:
                            nc.gpsimd.tensor_add(out=ysl, in0=ysl,
                                                 in1=yterm[:])
                nc.vector.scalar_tensor_tensor(
                    out=y_sb[:], in0=st["xi"][:, t0:t0 + HW],
                    scalar=W["dpd"][:], op0=Alu.mult,
                    in1=y_sb[:], op1=Alu.add)
                yg = evac.tile([128, HW], F16, tag="yg")
                nc.vector.tensor_mul(out=yg[:], in0=y_sb[:],
                                     in1=st["sz"][:, t0: t0 + HW])
                ft0 = L - (h + 1) * HW
                for g in range(NG):
                    for q in range(2):
                        op_ps = ps_mm.tile([128, CH], F32, tag="mm")
                        mm(op_ps[:], lhsT=W["wo"][:, g * 128:(g + 1) * 128],
                           rhs=yg[:, q * CH:(q + 1) * CH],
                           start=True, stop=True)
                        og = evac.tile([128, CH], F16, tag="og")
                        nc.scalar.copy(out=og[:, ::-1], in_=op_ps[:])
                        nc.sync.dma_start(
                            out=op_in[i][b].ap()[
                                g * 128:(g + 1) * 128,
                                ft0 + (1 - q) * CH: ft0 + (2 - q) * CH],
                            in_=og[:])
                        nc.gpsimd.collective_compute(
                "AllReduce", Alu.add, replica_groups=RG,
                ins=[op_in[i][b].ap()], outs=[op_out[i][b].ap()])

        def make_ab_items(ii, bb):
            def mk_start():
                if bb == 0:
                    Wq[ii] = load_weights(ii)
                stq[(ii, bb)] = ab_start(bb)
                a_chunk(ii, bb, Wq[ii], stq[(ii, bb)], 0)
            items = [mk_start]
            for c in (1, 2):
                items.append(lambda c=c: a_chunk(ii, bb, Wq[ii],
                                                 stq[(ii, bb)], c))

            def a3_rsqrt():
                a_chunk(ii, bb, Wq[ii], stq[(ii, bb)], 3)
                rsqrt_batch(stq[(ii, bb)])
            items.append(a3_rsqrt)
            for c in range(NCH):
                items.append(lambda c=c: b_chunk(ii, bb, Wq[ii],
                                                 stq[(ii, bb)], c))
            items.append(lambda: cwx(ii, bb, Wq[ii], stq[(ii, bb)]))
            return items

        SLOT9 = [(0, 6), (0, 9), (0, 12), (0, 15), (1, 2), (1, 5),
                 (1, 8), (1, 11), (1, 13)]
        SLOT4 = [(0, 8), (0, 13), (1, 3), (1, 8)]

        # prologue: block 0 batch 0 plain; batch 1 interleaves into D(0,0)
        Wq[0] = load_weights(0)
        stq[(0, 0)] = ab_start(0)
        for c in range(NCH):
            a_chunk(0, 0, Wq[0], stq[(0, 0)], c)
        rsqrt_batch(stq[(0, 0)])
        for c in range(NCH):
            b_chunk(0, 0, Wq[0], stq[(0, 0)], c)
        cwx(0, 0, Wq[0], stq[(0, 0)])

        for i in range(NB):
            items = make_ab_items(i, 1)
            sched = dict(zip(SLOT9, items))
            phase_d(i, 0, sched)
            # D(i, 1): interleave AB(i+1, 0), or the final-LN b=0 chunks
            if i + 1 < NB:
                items = make_ab_items(i + 1, 0)
                sched = dict(zip(SLOT9, items))
            else:
                items = [lambda c=c: fln_chunk(0, c) for c in range(NCH)]
                sched = dict(zip(SLOT4, items))
            phase_d(i, 1, sched)

        for c in range(NCH):
            fln_chunk(1, c)

def _host_prep(inputs):
    bf = np.float16
    x = np.asarray(inputs["x"], np.float32)
    ln_w = np.asarray(inputs["ln_w"], np.float32)
    ln_b = np.asarray(inputs["ln_b"], np.float32)
    W_in = np.asarray(inputs["W_in"], np.float32)
    conv_w = np.asarray(inputs["conv_w"], np.float32)
    conv_b = np.asarray(inputs["conv_b"], np.float32)
    W_x = np.asarray(inputs["W_x"], np.float32)
    W_dt = np.asarray(inputs["W_dt"], np.float32)
    b_dt = np.asarray(inputs["b_dt"], np.float32)
    A_log = np.asarray(inputs["A_log"], np.float32)
    D_p = np.asarray(inputs["D_p"], np.float32)
    W_out = np.asarray(inputs["W_out"], np.float32)
    normf_w = np.asarray(inputs["normf_w"], np.float32)
    normf_b = np.asarray(inputs["normf_b"], np.float32)

    xT = np.ascontiguousarray(x.transpose(2, 0, 1).reshape(D, T)).astype(bf)
    A = -np.exp(A_log)  # (NB, DI, NST)

    selbc = np.zeros((64, 32 * 128), np.float32)
    for q in range(32):
        selbc[32 + q, q * 128:(q + 1) * 128] = 1.0
    selbc = selbc.astype(bf)
    in_maps = []
    for k in range(NCORES):
        sl = slice(DS * k, DS * (k + 1))
        wi_arr = np.zeros((NB, 128, 1024), np.float32)
        negrs_arr = np.zeros((NB, 1, 256), np.float32)
        biasin_arr = np.zeros((NB, 128, 2), np.float32)
        convd_arr = np.zeros((NB, 128, KCONV * 128), np.float32)
        convb_arr = np.zeros((NB, 128, 1), np.float32)
        wx_arr = np.zeros((NB, 128, 64), np.float32)
        wdt_arr = np.zeros((NB, 32, 128), np.float32)
        bdt_arr = np.zeros((NB, 128, 1), np.float32)
        acols_arr = np.zeros((NB, 128, NST), np.float32)
        dpd_arr = np.zeros((NB, 128, 1), np.float32)
        wo_arr = np.zeros((NB, 128, 512), np.float32)
        for i in range(NB):
            Wf = W_in[i] * ln_w[i][None, :]          # (2DI, D)
            rows = [np.arange(DS * k, DS * (k + 1)),
                    np.arange(DI + DS * k, DI + DS * (k + 1))]
            for grp in range(2):
                Wg = Wf[rows[grp], :]                # (128, 512)
                lhsT = Wg.T.reshape(4, 128, 128)     # [kc, p, m]
                for kc in range(4):
                    wi_arr[i, :, (grp * 4 + kc) * 128:
                           (grp * 4 + kc + 1) * 128] = lhsT[kc]
                negrs_arr[i, 0, grp * 128:(grp + 1) * 128] = -Wg.sum(1) / D
                biasin_arr[i, :, grp] = W_in[i][rows[grp], :] @ ln_b[i]
            for kk in range(KCONV):
                np.fill_diagonal(
                    convd_arr[i, :, kk * 128:(kk + 1) * 128],
                    conv_w[i, sl, kk])
            convb_arr[i, :, 0] = conv_b[i, sl]
            wx_arr[i] = W_x[i][:, sl].T              # (128, 64)
            wdt_arr[i] = W_dt[i][sl, :].T            # (32, 128)
            bdt_arr[i, :, 0] = b_dt[i, sl]
            acols_arr[i] = A[i, sl, :]
            dpd_arr[i, :, 0] = D_p[i, sl]
            wo_arr[i] = W_out[i][:, sl].T            # (128, 512)
        in_maps.append({
            "xT": xT,
            "wi": wi_arr.astype(bf), "negrs": negrs_arr.astype(bf),
            "biasin": biasin_arr,
            "convd": convd_arr.astype(bf), "convb": convb_arr,
            "wxT": wx_arr.astype(bf), "wdtT": wdt_arr.astype(bf),
            "bdt": bdt_arr,
            "acols": acols_arr, "dpd": dpd_arr,
            "woT": wo_arr.astype(bf),
            "nfw": np.ascontiguousarray(normf_w.reshape(NG, 128).T),
            "nfb": np.ascontiguousarray(normf_b.reshape(NG, 128).T),
            "identin": np.eye(128, dtype=np.float32).astype(bf),
            "selbc": selbc,
            "onesin": np.ones((128, 128), np.float32).astype(bf),
        })
    has_lnb = bool(np.any(ln_b != 0.0))
    has_nfb = bool(np.any(normf_b != 0.0))
    return in_maps, has_lnb, has_nfb


def _get_program(has_lnb, has_nfb):
    key = (has_lnb, has_nfb)
    if key not in _PROGRAM_CACHE:
        _PROGRAM_CACHE[key] = _build_program(has_lnb, has_nfb)
    return _PROGRAM_CACHE[key]


LAST_RESULT = None


def kernel(**inputs) -> np.ndarray:
    global LAST_RESULT
    in_maps, has_lnb, has_nfb = _host_prep(inputs)
    nc = _get_program(has_lnb, has_nfb)
    res = bass_utils.run_bass_kernel_spmd(nc, in_maps,
                                          core_ids=list(range(NCORES)),
                                          trace=bool(os.environ.get("KTRACE")),
                                          tmpdir=os.environ.get("KTRACE_DIR"))
    LAST_RESULT = res
    out_T = res.results[0]["outT"]                   # (512, 4096)
    out = out_T.reshape(D, B, L).transpose(1, 2, 0)  # (B, L, D)
    return np.ascontiguousarray(out.astype(np.float32))


# revision 28
# speedup vs baseline: 2.6937x; 2.6937x over previous
"""Trainium2 Bass kernel for the bidirectional Mamba MixerModel problem.

fp16 on everything matmuls touch (weights + activations + the out-projection
AllReduce); fp32 kept on the precision-critical paths (softplus/dt chain, LN
stats, PSUM accumulators, x_dbl AllReduce).  The selective scan runs as
merged 1024-token tensor_tensor_scan ops per state with tiny fp32 carries;
the per-state C-multiply runs on the otherwise-idle GpSimd engine (PSUM
evacuated to SBUF by the Scalar engine first, since GpSimd has no PSUM
port).  Emission is software-pipelined: the next block/batch's LayerNorm +
in-projection + conv chunks are interleaved into the scan loop at fixed
(half, state) slots so the in-order engine queues overlap them with the
scan, with dedicated PSUM pools per phase so bank reuse can't serialize.
The inter-block sequence flip is folded into reversed write APs of the
out-projection evacuation.

Sharding: tensor-parallel over d_inner (128 channels per core, 8 cores),
AllReduce for x_dbl (fp32) and the out-projection partials (fp16) per
block/batch.
"""
import os
import sys
import numpy as np

sys.path.insert(0, "/opt/trn_rl_repo")

import ml_dtypes  # noqa: E402

import concourse.bass as bass  # noqa: E402,F401
import concourse.bacc as bacc  # noqa: E402
import concourse.tile as tile  # noqa: E402
from concourse import mybir  # noqa: E402
from concourse import bass_utils  # noqa: E402

F32 = mybir.dt.float32
F16 = mybir.dt.float16
F32R = mybir.dt.float32r
Alu = mybir.AluOpType
Act = mybir.ActivationFunctionType

B, L, D, DI = 2, 2048, 512, 1024
NST, KCONV, RDT, NB = 16, 4, 32, 4
NCORES = 8
DS = DI // NCORES          # 128 channels per core
T = B * L                  # 4096 tokens
CH = 512                   # token chunk for LN/in-proj (1 PSUM bank fp32)
NCH = L // CH              # 4 chunks per batch
HW = 1024                  # token half for the scan phase
NHW = L // HW              # 2 halves per batch
NG = D // 128              # 4 partition groups of the model dim
EPS = 1e-5
POOL_SCAN = False          # Pool-engine scan fails the TRN2 ISA opcode check

_PROGRAM_CACHE = {}


def _build_program(has_lnb: bool, has_nfb: bool):
    nc = bacc.Bacc("TRN2", target_bir_lowering=False, debug=False,
                   enable_asserts=False, num_devices=NCORES)

    tensors = {}
    tensors["xT"] = nc.dram_tensor("xT", [D, T], F16, kind="ExternalInput")
    tensors["wi"] = nc.dram_tensor("wi", [NB, 128, 1024], F16,
                                   kind="ExternalInput")
    tensors["negrs"] = nc.dram_tensor("negrs", [NB, 1, 256], F16,
                                      kind="ExternalInput")
    tensors["biasin"] = nc.dram_tensor("biasin", [NB, 128, 2], F32,
                                       kind="ExternalInput")
    tensors["convd"] = nc.dram_tensor("convd", [NB, 128, KCONV * 128], F16,
                                      kind="ExternalInput")
    tensors["convb"] = nc.dram_tensor("convb", [NB, 128, 1], F32,
                                      kind="ExternalInput")
    tensors["wxT"] = nc.dram_tensor("wxT", [NB, 128, 64], F16,
                                    kind="ExternalInput")
    tensors["wdtT"] = nc.dram_tensor("wdtT", [NB, 32, 128], F16,
                                     kind="ExternalInput")
    tensors["bdt"] = nc.dram_tensor("bdt", [NB, 128, 1], F32,
                                    kind="ExternalInput")
    tensors["acols"] = nc.dram_tensor("acols", [NB, 128, NST], F32,
                                      kind="ExternalInput")
    tensors["dpd"] = nc.dram_tensor("dpd", [NB, 128, 128], F16,
                                    kind="ExternalInput")
    tensors["woT"] = nc.dram_tensor("woT", [NB, 128, 512], F16,
                                    kind="ExternalInput")
    tensors["nfw"] = nc.dram_tensor("nfw", [128, NG], F32,
                                    kind="ExternalInput")
    tensors["nfb"] = nc.dram_tensor("nfb", [128, NG], F32,
                                    kind="ExternalInput")
    tensors["identin"] = nc.dram_tensor("identin", [128, 128], F16,
                                        kind="ExternalInput")
    tensors["selbc"] = nc.dram_tensor("selbc", [64, 32 * 128], F16,
                                      kind="ExternalInput")
    tensors["onesin"] = nc.dram_tensor("onesin", [128, 128], F16,
                                       kind="ExternalInput")
    tensors["outT"] = nc.dram_tensor("outT", [D, T], F32,
                                     kind="ExternalOutput")

    xdbl_in, xdbl_out, op_in, op_out = [], [], [], []
    for i in range(NB):
        xi_b, xo_b, oi_b, oo_b = [], [], [], []
        for b in range(B):
            xi_b.append(nc.dram_tensor(f"xdbl_in_{i}_{b}", [64, L], F32,
                                       kind="Internal"))
            xo_b.append(nc.dram_tensor(f"xdbl_out_{i}_{b}", [64, L], F32,
                                       kind="Internal", addr_space="Shared"))
            oi_b.append(nc.dram_tensor(f"op_in_{i}_{b}", [D, L], F16,
                                       kind="Internal"))
            oo_b.append(nc.dram_tensor(f"op_out_{i}_{b}", [D, L], F16,
                                       kind="Internal", addr_space="Shared"))
        xdbl_in.append(xi_b); xdbl_out.append(xo_b)
        op_in.append(oi_b); op_out.append(oo_b)
    tensors["xdbl_in"], tensors["xdbl_out"] = xdbl_in, xdbl_out
    tensors["op_in"], tensors["op_out"] = op_in, op_out

    with tile.TileContext(nc) as tc:
        _emit(nc, tc, tensors, has_lnb, has_nfb)

    nc.compile()
    return nc


def _emit(nc, tc, Tn, has_lnb, has_nfb):
    import contextlib
    RG = [list(range(NCORES))]
    xdbl_in, xdbl_out = Tn["xdbl_in"], Tn["xdbl_out"]
    op_in, op_out = Tn["op_in"], Tn["op_out"]

    ctx = contextlib.ExitStack()
    with ctx:
        consts = ctx.enter_context(tc.tile_pool(name="consts", bufs=1))
        wpool = ctx.enter_context(tc.tile_pool(name="wpool", bufs=2))
        xin = ctx.enter_context(tc.tile_pool(name="xin", bufs=5))
        small = ctx.enter_context(tc.tile_pool(name="small", bufs=2))
        stats = ctx.enter_context(tc.tile_pool(name="stats", bufs=5))
        bigs = ctx.enter_context(tc.tile_pool(name="bigs", bufs=1))
        spool = ctx.enter_context(tc.tile_pool(name="spool", bufs=3))
        evac = ctx.enter_context(tc.tile_pool(name="evac", bufs=2))
        # PSUM 8 banks: ab(2) for phases A/B/wx/final-LN, mm(2) for dt/op,
        # bc(2) for B/C broadcasts, y(2) for the scan accumulator.  Separate
        # pools keep next-block stats from serializing behind the scan phase.
        ps_ab = ctx.enter_context(tc.tile_pool(name="ps_ab", bufs=2,
                                               space="PSUM"))
        ps_mm = ctx.enter_context(tc.tile_pool(name="ps_mm", bufs=2,
                                               space="PSUM"))
        ps_bc = ctx.enter_context(tc.tile_pool(name="ps_bc", bufs=2,
                                               space="PSUM"))
        ps_y = ctx.enter_context(tc.tile_pool(name="ps_y", bufs=1,
                                              space="PSUM"))

        ident = consts.tile([128, 128], F16, tag="ident")
        nc.sync.dma_start(out=ident[:], in_=Tn["identin"].ap())
        onesbf = consts.tile([128, 128], F16, tag="onesbf")
        nc.sync.dma_start(out=onesbf[:], in_=Tn["onesin"].ap())
        ones1r = consts.tile([1, 128], F32R, tag="ones1r")
        nc.vector.memset(ones1r[:].bitcast(F32), 1.0)
        nfw_sb = consts.tile([128, NG], F32, tag="nfw")
        nc.sync.dma_start(out=nfw_sb[:], in_=Tn["nfw"].ap())
        nfb_sb = consts.tile([128, NG], F32, tag="nfb")
        nc.sync.dma_start(out=nfb_sb[:], in_=Tn["nfb"].ap())
        eps_sb = consts.tile([128, 1], F32, tag="eps")
        nc.vector.memset(eps_sb[:], EPS)
        selbc_sb = consts.tile([64, 32 * 128], F16, tag="selbc")
        nc.sync.dma_start(out=selbc_sb[:], in_=Tn["selbc"].ap())

        onescol = onesbf[:, 0:1]     # [128,1] bf16 lhsT for stats
        ones1 = onesbf[0:1, :]       # [1,128] bf16 lhsT for broadcasts

        def mm(out, lhsT, rhs, **kw):
            nc.tensor.matmul(out, lhsT=lhsT, rhs=rhs, **kw)

        def src_ap(i, b, g, t0, t1):
            """Block-i input (already flipped), batch b, feature group g."""
            if i == 0:
                return Tn["xT"].ap()[128 * g:128 * (g + 1),
                                     b * L + t0: b * L + t1]
            return op_out[i - 1][b].ap()[128 * g:128 * (g + 1), t0:t1]

        def ln_stats(st_ps, xg_tiles):
            # st_ps: [128, CH] psum tile; partition 0 = sum x, 32 = sum x^2
            for g in range(NG):
                xsq = small.tile([128, CH], F16, tag="xsq")
                nc.scalar.square(out=xsq[:], in_=xg_tiles[g][:])
                mm(st_ps[0:1, :], lhsT=onescol, rhs=xg_tiles[g][:],
                   start=(g == 0), stop=(g == NG - 1), skip_group_check=True)
                mm(st_ps[32:33, :], lhsT=onescol, rhs=xsq[:],
                   start=(g == 0), stop=(g == NG - 1), skip_group_check=True)

        def load_weights(i):
            W = {}
            W["wi"] = wpool.tile([128, 1024], F16, tag="wi", name="wi")
            nc.sync.dma_start(out=W["wi"][:], in_=Tn["wi"].ap()[i])
            W["negrs"] = wpool.tile([1, 256], F16, tag="negrs", name="negrs")
            nc.sync.dma_start(out=W["negrs"][:], in_=Tn["negrs"].ap()[i])
            W["convd"] = wpool.tile([128, KCONV * 128], F16, tag="convd", name="convd")
            nc.sync.dma_start(out=W["convd"][:], in_=Tn["convd"].ap()[i])
            W["convb"] = wpool.tile([128, 1], F32, tag="convb", name="convb")
            nc.sync.dma_start(out=W["convb"][:], in_=Tn["convb"].ap()[i])
            W["wx"] = wpool.tile([128, 64], F16, tag="wx", name="wx")
            nc.sync.dma_start(out=W["wx"][:], in_=Tn["wxT"].ap()[i])
            W["wdt"] = wpool.tile([32, 128], F16, tag="wdt", name="wdt")
            nc.sync.dma_start(out=W["wdt"][:], in_=Tn["wdtT"].ap()[i])
            W["bdt"] = wpool.tile([128, 1], F32, tag="bdt", name="bdt")
            nc.sync.dma_start(out=W["bdt"][:], in_=Tn["bdt"].ap()[i])
            W["acols"] = wpool.tile([128, NST], F32, tag="acols", name="acols")
            nc.sync.dma_start(out=W["acols"][:], in_=Tn["acols"].ap()[i])
            W["dpd"] = wpool.tile([128, 128], F16, tag="dpd", name="dpd")
            nc.sync.dma_start(out=W["dpd"][:], in_=Tn["dpd"].ap()[i])
            W["wo"] = wpool.tile([128, 512], F16, tag="wo", name="wo")
            nc.sync.dma_start(out=W["wo"][:], in_=Tn["woT"].ap()[i])
            if has_lnb:
                W["biasin"] = wpool.tile([128, 2], F32, tag="biasin", name="biasin")
                nc.sync.dma_start(out=W["biasin"][:],
                                  in_=Tn["biasin"].ap()[i])
            return W

        def ab_start(b):
            st = {}
            st["xipad"] = bigs.tile([128, L + 3], F16, tag=f"xipad{b}",
                                    name=f"xipad{b}")
            st["xi"] = bigs.tile([128, L], F16, tag=f"xibf{b}",
                                 name=f"xibf{b}")
            st["sz"] = bigs.tile([128, L], F16, tag=f"sz{b}", name=f"sz{b}")
            st["varall"] = stats.tile([128, CH], F32, tag="varall",
                                      name="varall", bufs=2)
            st["rstd"] = stats.tile([128, CH], F16, tag="rstdall",
                                    name="rstdall", bufs=2)
            st["s1"] = {}
            nc.vector.memset(st["varall"][:], 1.0)
            nc.vector.memset(st["xipad"][:, 0:3], 0.0)
            return st

        def a_chunk(i, b, W, st, c):
            t0 = c * CH
            xg_tiles = []
            for g in range(NG):
                xg = xin.tile([128, CH], F16, tag="xg")
                nc.sync.dma_start(out=xg[:],
                                  in_=src_ap(i, b, g, t0, t0 + CH))
                xg_tiles.append(xg)
            st_ps = ps_ab.tile([128, CH], F32, tag="ab")
            ln_stats(st_ps, xg_tiles)
            s1_row = stats.tile([1, CH], F16, tag="s1", name="s1_row")
            nc.scalar.copy(out=s1_row[:], in_=st_ps[0:1, :])
            st["s1"][c] = s1_row
            mu2 = small.tile([1, CH], F32, tag="mu2")
            nc.scalar.activation(out=mu2[:], in_=st_ps[0:1, :],
                                 func=Act.Square, scale=1.0 / D)
            nc.vector.scalar_tensor_tensor(
                out=st["varall"][32 * c:32 * c + 1, :], in0=st_ps[32:33, :],
                scalar=1.0 / D, in1=mu2[:], op0=Alu.mult, op1=Alu.subtract)

        def rsqrt_batch(st):
            # one Ln-table round-trip for all four chunks' rstd rows
            nc.scalar.activation(out=st["varall"][:], in_=st["varall"][:],
                                 func=Act.Ln, bias=eps_sb[:])
            nc.scalar.activation(out=st["rstd"][:], in_=st["varall"][:],
                                 func=Act.Exp, scale=-0.5)

        def b_chunk(i, b, W, st, c):
            t0 = c * CH
            xg_tiles = []
            for g in range(NG):
                xg = xin.tile([128, CH], F16, tag="xg")
                nc.sync.dma_start(out=xg[:],
                                  in_=src_ap(i, b, g, t0, t0 + CH))
                xg_tiles.append(xg)
            rstd_row = small.tile([1, CH], F16, tag="rstds",
                                  name="rstd_row")
            nc.scalar.copy(out=rstd_row[:],
                           in_=st["rstd"][32 * c:32 * c + 1, :])
            rbc_ps = ps_ab.tile([128, CH], F32, tag="ab")
            mm(rbc_ps[:], lhsT=ones1, rhs=rstd_row[:], start=True, stop=True)
            rbc = small.tile([128, CH], F16, tag="rbc")
            nc.scalar.copy(out=rbc[:], in_=rbc_ps[:])
            for grp in range(2):  # 0 = xi, 1 = z
                xz_ps = ps_ab.tile([128, CH], F32, tag="ab")
                for k in range(4):
                    lh = W["wi"][:, (grp * 4 + k) * 128:
                                 (grp * 4 + k + 1) * 128]
                    mm(xz_ps[:], lhsT=lh, rhs=xg_tiles[k][:],
                       start=(k == 0), stop=False)
                mm(xz_ps[:], lhsT=W["negrs"][:, grp * 128:(grp + 1) * 128],
                   rhs=st["s1"][c][:], start=False, stop=True)
                if grp == 0:
                    dest = st["xipad"][:, 3 + t0: 3 + t0 + CH]
                else:
                    dest = st["sz"][:, t0: t0 + CH]
                nc.vector.tensor_mul(out=dest, in0=xz_ps[:], in1=rbc[:])
                if has_lnb:
                    nc.vector.tensor_scalar_add(
                        out=dest, in0=dest,
                        scalar1=W["biasin"][:, grp:grp + 1])
            cv_ps = ps_ab.tile([128, CH], F32, tag="ab")
            for kk in range(KCONV):
                mm(cv_ps[:], lhsT=W["convd"][:, kk * 128:(kk + 1) * 128],
                   rhs=st["xipad"][:, t0 + kk: t0 + kk + CH],
                   start=(kk == 0), stop=(kk == KCONV - 1))
            nc.scalar.activation(out=st["xi"][:, t0:t0 + CH], in_=cv_ps[:],
                                 func=Act.Identity, bias=W["convb"][:])

        def cwx(i, b, W, st):
            """silu + Wx projection + x_dbl AllReduce for one batch."""
            nc.scalar.activation(out=st["xi"][:], in_=st["xi"][:],
                                 func=Act.Silu)
            nc.scalar.activation(out=st["sz"][:], in_=st["sz"][:],
                                 func=Act.Silu)
            for c in range(NCH):
                t0 = c * CH
                wx_ps = ps_ab.tile([128, CH], F32, tag="ab")
                mm(wx_ps[0:64, :], lhsT=W["wx"][:],
                   rhs=st["xi"][:, t0:t0 + CH], start=True, stop=True,
                   skip_group_check=True)
                wxe = small.tile([64, CH], F32, tag="wxe", name="wxe")
                nc.scalar.copy(out=wxe[:], in_=wx_ps[0:64, :])
                nc.sync.dma_start(out=xdbl_in[i][b].ap()[:, t0:t0 + CH],
                                  in_=wxe[:])
            nc.gpsimd.collective_compute(
                "AllReduce", Alu.add, replica_groups=RG,
                ins=[xdbl_in[i][b].ap()], outs=[xdbl_out[i][b].ap()])

        def fln_chunk(b, c):
            """Final layernorm for one 512-token chunk."""
            t0 = c * CH
            xg_tiles = []
            for g in range(NG):
                xg = xin.tile([128, CH], F16, tag="xg")
                nc.sync.dma_start(out=xg[:],
                                  in_=src_ap(NB, b, g, t0, t0 + CH))
                xg_tiles.append(xg)
            st_ps = ps_ab.tile([128, CH], F32, tag="ab")
            ln_stats(st_ps, xg_tiles)
            m_row = small.tile([1, CH], F32R, tag="m_row")
            nc.vector.tensor_scalar_mul(out=m_row[:], in0=st_ps[0:1, :],
                                        scalar1=1.0 / D)
            mu2 = small.tile([1, CH], F32, tag="mu2")
            nc.vector.tensor_mul(out=mu2[:], in0=m_row[:].bitcast(F32),
                                 in1=m_row[:].bitcast(F32))
            var_row = small.tile([1, CH], F32, tag="var")
            nc.vector.scalar_tensor_tensor(
                out=var_row[:], in0=st_ps[32:33, :], scalar=1.0 / D,
                in1=mu2[:], op0=Alu.mult, op1=Alu.subtract)
            rstd_row = small.tile([1, CH], F32R, tag="rstdf",
                                  name="rstd_row")
            nc.scalar.activation(out=var_row[:], in_=var_row[:],
                                 func=Act.Ln, bias=eps_sb[:1, :])
            nc.scalar.activation(out=rstd_row[:], in_=var_row[:],
                                 func=Act.Exp, scale=-0.5)
            mbc_ps = ps_ab.tile([128, CH], F32, tag="ab")
            mm(mbc_ps[:], lhsT=ones1r[:], rhs=m_row[:], start=True, stop=True)
            rbc_ps = ps_ab.tile([128, CH], F32, tag="ab")
            mm(rbc_ps[:], lhsT=ones1r[:], rhs=rstd_row[:],
               start=True, stop=True)
            rbc = small.tile([128, CH], F32, tag="rbcf")
            nc.scalar.copy(out=rbc[:], in_=rbc_ps[:])
            for g in range(NG):
                t1_sb = small.tile([128, CH], F32, tag="xsqf", name="t1_sb")
                nc.vector.tensor_sub(out=t1_sb[:], in0=xg_tiles[g][:],
                                     in1=mbc_ps[:])
                o_sb = evac.tile([128, CH], F32, tag="ogf", name="o_sb")
                nc.vector.scalar_tensor_tensor(
                    out=o_sb[:], in0=t1_sb[:], scalar=nfw_sb[:, g:g + 1],
                    in1=rbc[:], op0=Alu.mult, op1=Alu.mult)
                if has_nfb:
                    nc.vector.tensor_scalar_add(
                        out=o_sb[:], in0=o_sb[:], scalar1=nfb_sb[:, g:g + 1])
                nc.sync.dma_start(
                    out=Tn["outT"].ap()[g * 128:(g + 1) * 128,
                                        b * L + t0: b * L + t0 + CH],
                    in_=o_sb[:])

        Wq = {}
        stq = {}

        def phase_d(i, b, sched):
            """Softplus dt + merged scans + out-proj for one batch, with
            pending next-phase work interleaved at fixed (h, n) slots."""
            W, st = Wq[i], stq[(i, b)]
            xdbl_sb = bigs.tile([64, L], F32, tag=f"xdbl{b}",
                                name=f"xdbl{b}")
            nc.sync.dma_start(out=xdbl_sb[:], in_=xdbl_out[i][b].ap())
            xdbl_bf = bigs.tile([64, L], F16, tag=f"xdblbf{b}",
                                name=f"xdblbf{b}")
            nc.scalar.copy(out=xdbl_bf[:], in_=xdbl_sb[:])

            dt = bigs.tile([128, L], F32, tag=f"dt{b}", name=f"dt{b}")
            for c in range(NCH):
                t0 = c * CH
                dt_ps = ps_mm.tile([128, CH], F32, tag="mm", name="dt_ps")
                mm(dt_ps[:], lhsT=W["wdt"][:],
                   rhs=xdbl_bf[0:32, t0:t0 + CH], start=True, stop=True)
                nc.scalar.activation(out=dt[:, t0:t0 + CH], in_=dt_ps[:],
                                     func=Act.Identity, bias=W["bdt"][:])
            esb = bigs.tile([128, L], F32, tag=f"esb{b}", name=f"esb{b}")
            nc.scalar.activation(out=esb[:], in_=dt[:], func=Act.Exp,
                                 scale=-1.0)
            nc.scalar.activation(out=esb[:], in_=esb[:], func=Act.Ln,
                                 bias=1.0)
            nc.vector.tensor_add(out=dt[:], in0=dt[:], in1=esb[:])
            dtx = bigs.tile([128, L], F16, tag=f"dtx{b}", name=f"dtx{b}")
            nc.vector.tensor_mul(out=dtx[:], in0=dt[:], in1=st["xi"][:])

            carry = small.tile([128, NST], F32, tag="carry")
            for h in range(NHW):
                t0 = h * HW
                y_ps = ps_y.tile([128, HW], F32, tag="y")
                for n in range(NST):
                    if (h, n) in sched:
                        sched[(h, n)]()
                    dA = spool.tile([128, HW], F16, tag="dA")
                    nc.scalar.activation(out=dA[:], in_=dt[:, t0:t0 + HW],
                                         func=Act.Exp,
                                         scale=W["acols"][:, n:n + 1])
                    dBu = spool.tile([128, HW], F16, tag="dBu")
                    for q in range(2):
                        bbc_ps = ps_bc.tile([128, CH], F32, tag="bc")
                        mm(bbc_ps[:],
                           lhsT=selbc_sb[:, n * 128:(n + 1) * 128],
                           rhs=xdbl_bf[:, t0 + q * CH: t0 + (q + 1) * CH],
                           start=True, stop=True)
                        nc.vector.tensor_mul(
                            out=dBu[:, q * CH:(q + 1) * CH],
                            in0=dtx[:, t0 + q * CH: t0 + (q + 1) * CH],
                            in1=bbc_ps[:])
                    hsc = spool.tile([128, HW], F16, tag="h")
                    init = 0.0 if h == 0 else carry[:, n:n + 1]
                    nc.vector.tensor_tensor_scan(hsc[:], dA[:], dBu[:],
                                                 init, op0=Alu.mult,
                                                 op1=Alu.add)
                    if h < NHW - 1:
                        nc.scalar.copy(out=carry[:, n:n + 1],
                                       in_=hsc[:, HW - 1:HW])
                    for q in range(2):
                        cbc_ps = ps_bc.tile([128, CH], F32, tag="bc")
                        mm(cbc_ps[:],
                           lhsT=selbc_sb[:, (16 + n) * 128:(17 + n) * 128],
                           rhs=xdbl_bf[:, t0 + q * CH: t0 + (q + 1) * CH],
                           start=True, stop=True)
                        cbc_sb = spool.tile([128, CH], F16, tag="cbcsb")
                        nc.scalar.copy(out=cbc_sb[:], in_=cbc_ps[:])
                        yterm = spool.tile([128, CH], F16, tag="yterm")
                        nc.gpsimd.tensor_mul(
                            out=yterm[:], in0=hsc[:, q * CH:(q + 1) * CH],
                            in1=cbc_sb[:])
                        mm(y_ps[:, q * CH:(q + 1) * CH], lhsT=ident[:],
                           rhs=yterm[:], start=(n == 0), stop=False,
                           skip_group_check=True)
                for q in range(2):
                    mm(y_ps[:, q * CH:(q + 1) * CH], lhsT=W["dpd"][:],
                       rhs=st["xi"][:, t0 + q * CH: t0 + (q + 1) * CH],
                       start=False, stop=True, skip_group_check=True)
                yg = evac.tile([128, HW], F16, tag="yg")
                nc.vector.tensor_mul(out=yg[:], in0=y_ps[:],
                                     in1=st["sz"][:, t0: t0 + HW])
                ft0 = L - (h + 1) * HW
                for g in range(NG):
                    for q in range(2):
                        op_ps = ps_mm.tile([128, CH], F32, tag="mm")
                        mm(op_ps[:], lhsT=W["wo"][:, g * 128:(g + 1) * 128],
                           rhs=yg[:, q * CH:(q + 1) * CH],
                           start=True, stop=True)
                        og = evac.tile([128, CH], F16, tag="og")
                        nc.scalar.copy(out=og[:, ::-1], in_=op_ps[:])
                        nc.sync.dma_start(
                            out=op_in[i][b].ap()[
                                g * 128:(g + 1) * 128,
                                ft0 + (1 - q) * CH: ft0 + (2 - q) * CH],
                            in_=og[:])
            nc.gpsimd.collective_compute(
                "AllReduce", Alu.add, replica_groups=RG,
                ins=[op_in[i][b].ap()], outs=[op_out[i][b].ap()])

        def make_ab_items(ii, bb):
            def mk_start():
                if bb == 0:
                    Wq[ii] = load_weights(ii)
                stq[(ii, bb)] = ab_start(bb)
                a_chunk(ii, bb, Wq[ii], stq[(ii, bb)], 0)
            items = [mk_start]
            for c in (1, 2):
                items.append(lambda c=c: a_chunk(ii, bb, Wq[ii],
                                                 stq[(ii, bb)], c))

            def a3_rsqrt():
                a_chunk(ii, bb, Wq[ii], stq[(ii, bb)], 3)
                rsqrt_batch(stq[(ii, bb)])
            items.append(a3_rsqrt)
            for c in range(NCH):
                items.append(lambda c=c: b_chunk(ii, bb, Wq[ii],
                                                 stq[(ii, bb)], c))
            items.append(lambda: cwx(ii, bb, Wq[ii], stq[(ii, bb)]))
            return items

        SLOT9 = [(0, 6), (0, 9), (0, 12), (0, 15), (1, 2), (1, 5),
                 (1, 8), (1, 11), (1, 13)]
        SLOT4 = [(0, 8), (0, 13), (1, 3), (1, 8)]

        # prologue: block 0 batch 0 plain; batch 1 interleaves into D(0,0)
        Wq[0] = load_weights(0)
        stq[(0, 0)] = ab_start(0)
        for c in range(NCH):
            a_chunk(0, 0, Wq[0], stq[(0, 0)], c)
        rsqrt_batch(stq[(0, 0)])
        for c in range(NCH):
            b_chunk(0, 0, Wq[0], stq[(0, 0)], c)
        cwx(0, 0, Wq[0], stq[(0, 0)])

        for i in range(NB):
            items = make_ab_items(i, 1)
            sched = dict(zip(SLOT9, items))
            phase_d(i, 0, sched)
            # D(i, 1): interleave AB(i+1, 0), or the final-LN b=0 chunks
            if i + 1 < NB:
                items = make_ab_items(i + 1, 0)
                sched = dict(zip(SLOT9, items))
            else:
                items = [lambda c=c: fln_chunk(0, c) for c in range(NCH)]
                sched = dict(zip(SLOT4, items))
            phase_d(i, 1, sched)

        for c in range(NCH):
            fln_chunk(1, c)

def _host_prep(inputs):
    bf = np.float16
    x = np.asarray(inputs["x"], np.float32)
    ln_w = np.asarray(inputs["ln_w"], np.float32)
    ln_b = np.asarray(inputs["ln_b"], np.float32)
    W_in = np.asarray(inputs["W_in"], np.float32)
    conv_w = np.asarray(inputs["conv_w"], np.float32)
    conv_b = np.asarray(inputs["conv_b"], np.float32)
    W_x = np.asarray(inputs["W_x"], np.float32)
    W_dt = np.asarray(inputs["W_dt"], np.float32)
    b_dt = np.asarray(inputs["b_dt"], np.float32)
    A_log = np.asarray(inputs["A_log"], np.float32)
    D_p = np.asarray(inputs["D_p"], np.float32)
    W_out = np.asarray(inputs["W_out"], np.float32)
    normf_w = np.asarray(inputs["normf_w"], np.float32)
    normf_b = np.asarray(inputs["normf_b"], np.float32)

    xT = np.ascontiguousarray(x.transpose(2, 0, 1).reshape(D, T)).astype(bf)
    A = -np.exp(A_log)  # (NB, DI, NST)

    selbc = np.zeros((64, 32 * 128), np.float32)
    for q in range(32):
        selbc[32 + q, q * 128:(q + 1) * 128] = 1.0
    selbc = selbc.astype(bf)
    in_maps = []
    for k in range(NCORES):
        sl = slice(DS * k, DS * (k + 1))
        wi_arr = np.zeros((NB, 128, 1024), np.float32)
        negrs_arr = np.zeros((NB, 1, 256), np.float32)
        biasin_arr = np.zeros((NB, 128, 2), np.float32)
        convd_arr = np.zeros((NB, 128, KCONV * 128), np.float32)
        convb_arr = np.zeros((NB, 128, 1), np.float32)
        wx_arr = np.zeros((NB, 128, 64), np.float32)
        wdt_arr = np.zeros((NB, 32, 128), np.float32)
        bdt_arr = np.zeros((NB, 128, 1), np.float32)
        acols_arr = np.zeros((NB, 128, NST), np.float32)
        dpd_arr = np.zeros((NB, 128, 128), np.float32)
        wo_arr = np.zeros((NB, 128, 512), np.float32)
        for i in range(NB):
            Wf = W_in[i] * ln_w[i][None, :]          # (2DI, D)
            rows = [np.arange(DS * k, DS * (k + 1)),
                    np.arange(DI + DS * k, DI + DS * (k + 1))]
            for grp in range(2):
                Wg = Wf[rows[grp], :]                # (128, 512)
                lhsT = Wg.T.reshape(4, 128, 128)     # [kc, p, m]
                for kc in range(4):
                    wi_arr[i, :, (grp * 4 + kc) * 128:
                           (grp * 4 + kc + 1) * 128] = lhsT[kc]
                negrs_arr[i, 0, grp * 128:(grp + 1) * 128] = -Wg.sum(1) / D
                biasin_arr[i, :, grp] = W_in[i][rows[grp], :] @ ln_b[i]
            for kk in range(KCONV):
                np.fill_diagonal(
                    convd_arr[i, :, kk * 128:(kk + 1) * 128],
                    conv_w[i, sl, kk])
            convb_arr[i, :, 0] = conv_b[i, sl]
            wx_arr[i] = W_x[i][:, sl].T              # (128, 64)
            wdt_arr[i] = W_dt[i][sl, :].T            # (32, 128)
            bdt_arr[i, :, 0] = b_dt[i, sl]
            acols_arr[i] = A[i, sl, :]
            np.fill_diagonal(dpd_arr[i], D_p[i, sl])
            wo_arr[i] = W_out[i][:, sl].T            # (128, 512)
        in_maps.append({
            "xT": xT,
            "wi": wi_arr.astype(bf), "negrs": negrs_arr.astype(bf),
            "biasin": biasin_arr,
            "convd": convd_arr.astype(bf), "convb": convb_arr,
            "wxT": wx_arr.astype(bf), "wdtT": wdt_arr.astype(bf),
            "bdt": bdt_arr,
            "acols": acols_arr, "dpd": dpd_arr.astype(bf),
            "woT": wo_arr.astype(bf),
            "nfw": np.ascontiguousarray(normf_w.reshape(NG, 128).T),
            "nfb": np.ascontiguousarray(normf_b.reshape(NG, 128).T),
            "identin": np.eye(128, dtype=np.float32).astype(bf),
            "selbc": selbc,
            "onesin": np.ones((128, 128), np.float32).astype(bf),
        })
    has_lnb = bool(np.any(ln_b != 0.0))
    has_nfb = bool(np.any(normf_b != 0.0))
    return in_maps, has_lnb, has_nfb


def _get_program(has_lnb, has_nfb):
    key = (has_lnb, has_nfb)
    if key not in _PROGRAM_CACHE:
        _PROGRAM_CACHE[key] = _build_program(has_lnb, has_nfb)
    return _PROGRAM_CACHE[key]


LAST_RESULT = None


def kernel(**inputs) -> np.ndarray:
    global LAST_RESULT
    in_maps, has_lnb, has_nfb = _host_prep(inputs)
    nc = _get_program(has_lnb, has_nfb)
    res = bass_utils.run_bass_kernel_spmd(nc, in_maps,
                                          core_ids=list(range(NCORES)),
                                          trace=bool(os.environ.get("KTRACE")),
                                          tmpdir=os.environ.get("KTRACE_DIR"))
    LAST_RESULT = res
    out_T = res.results[0]["outT"]                   # (512, 4096)
    out = out_T.reshape(D, B, L).transpose(1, 2, 0)  # (B, L, D)
    return np.ascontiguousarray(out.astype(np.float32))
